# revision 1
# baseline (speedup 1.0000x reference)
"""CenterLoss Trainium2 kernel.

loss = mean_b clip(||x_b - centers[labels_b]||^2, 1e-12, 1e12)

Shapes (hardcoded): x [8192, 512] f32, labels [8192] int64 in [0, 10000),
centers [10000, 512] f32.  Output: f32 scalar.

Strategy: data-parallel over batch across 8 cores (1024 rows each);
centers stay in HBM (replicated input) and each core gathers exactly the
1024 rows it needs with one indirect DMA (labels as row offsets).  The
full [B, C] distmat of the reference is never formed - only the diagonal
entries distmat[b, labels_b] are needed, so the kernel is memory-bound:
~4 MB of HBM traffic per core (2 MB x-shard + 2 MB gathered centers).

Per-core layout: batch row r = p*8 + t maps to SBUF partition p, tile t
(8 tiles of [128, 512]).  This makes the label load a single contiguous
[128, 8] DMA and x a single [128, 4096] DMA (16 KB contiguous per
partition).  Per tile: DVE subtract then fused DVE multiply+row-reduce
(tensor_tensor_reduce) producing dist[p, t].  The [128, 8] per-row
distances go back to the host, which applies clip and the global mean.
"""

import sys

import numpy as np

try:
    import concourse  # noqa: F401
except ImportError:  # pragma: no cover
    sys.path.insert(0, "/opt/trn_rl_repo")

B, D, C = 8192, 512, 10000
N_CORES = 8
P = 128
ROWS = B // N_CORES  # 1024 rows per core
T = ROWS // P        # 8 tiles of 128 rows

CLAMP_MIN = 1e-12
CLAMP_MAX = 1e12

_CACHE = {}


def _build():
    import concourse.bacc as bacc
    import concourse.tile as tile
    from concourse import bass, mybir

    f32 = mybir.dt.float32
    i32 = mybir.dt.int32

    nc = bacc.Bacc("TRN2", target_bir_lowering=False, num_devices=N_CORES)
    x = nc.dram_tensor("x", [ROWS, D], f32, kind="ExternalInput")
    labels = nc.dram_tensor("labels", [ROWS, 1], i32, kind="ExternalInput")
    centers = nc.dram_tensor("centers", [C, D], f32, kind="ExternalInput")
    out = nc.dram_tensor("out", [P, T], f32, kind="ExternalOutput")

    with tile.TileContext(nc) as tc:
        with (
            tc.tile_pool(name="big", bufs=1) as big,
            tc.tile_pool(name="small", bufs=1) as small,
            tc.tile_pool(name="work", bufs=4) as work,
        ):
            idx = small.tile([P, T], i32)
            dist = small.tile([P, T], f32)
            xbig = big.tile([P, T * D], f32)
            cbig = big.tile([P, T * D], f32)

            # idx[p, t] = labels[p*T + t]; 32 B contiguous per partition.
            nc.sync.dma_start(
                out=idx[:], in_=labels[:, :].rearrange("(p t) o -> p (t o)", p=P)
            )
            # xbig[p, t*D:(t+1)*D] = x[p*T + t, :]; 16 KB contiguous per partition.
            nc.sync.dma_start(
                out=xbig[:], in_=x[:, :].rearrange("(p t) d -> p (t d)", p=P)
            )
            # cbig[p, t*D:(t+1)*D] = centers[idx[p, t], :].  One indirect
            # DMA per 128 rows (128 descriptors fits DynamicDMAScratch).
            for t in range(T):
                sl = slice(t * D, (t + 1) * D)
                nc.gpsimd.indirect_dma_start(
                    out=cbig[:, sl],
                    out_offset=None,
                    in_=centers[:, :],
                    in_offset=bass.IndirectOffsetOnAxis(ap=idx[:, t : t + 1], axis=0),
                )
            for t in range(T):
                sl = slice(t * D, (t + 1) * D)
                diff = work.tile([P, D], f32, tag="diff")
                sq = work.tile([P, D], f32, tag="sq")
                nc.vector.tensor_sub(diff[:], xbig[:, sl], cbig[:, sl])
                # sq = diff^2 on ACT; dist[:, t] = row-sum(sq) via ACT accum.
                # (tensor_tensor_reduce would fuse this on DVE but crashes the
                # device on this runtime; ACT also gives DVE/ACT pipelining.)
                nc.scalar.activation(
                    sq[:],
                    diff[:],
                    mybir.ActivationFunctionType.Square,
                    accum_out=dist[:, t : t + 1],
                )
            nc.sync.dma_start(out=out[:, :], in_=dist[:])

    nc.compile()
    return nc


def get_nc():
    nc = _CACHE.get("nc")
    if nc is None:
        nc = _CACHE["nc"] = _build()
    return nc


def make_in_maps(x, labels, centers):
    labels_i32 = np.ascontiguousarray(labels.astype(np.int32).reshape(B, 1))
    x = np.ascontiguousarray(x, dtype=np.float32)
    centers = np.ascontiguousarray(centers, dtype=np.float32)
    in_maps = []
    for i in range(N_CORES):
        lo, hi = i * ROWS, (i + 1) * ROWS
        in_maps.append(
            {"x": x[lo:hi], "labels": labels_i32[lo:hi], "centers": centers}
        )
    return in_maps


def finish(per_core_outs):
    """per_core_outs: list of 8 [P, T] arrays -> f32 scalar loss."""
    d = np.concatenate([np.asarray(o).reshape(-1) for o in per_core_outs])
    d = np.clip(d, CLAMP_MIN, CLAMP_MAX)
    return np.asarray(np.mean(d, dtype=np.float64), dtype=np.float32)


def kernel(x, labels, centers):
    from concourse.bass_utils import run_bass_kernel_spmd

    nc = get_nc()
    in_maps = make_in_maps(x, labels, centers)
    res = run_bass_kernel_spmd(nc, in_maps, core_ids=list(range(N_CORES)))
    return finish([r["out"] for r in res.results])

